# revision 24
# baseline (speedup 1.0000x reference)
"""Distributed GAT (3-layer, heads=1) Bass kernel for 8 TRN2 NeuronCores.

Strategy (dst-sharded, batched dma_gather over a bf16 pair-row table):
- Host: permute nodes by in-degree (excl. self-loop) into degree-homogeneous
  blocks of 128, deal blocks round-robin to 8 cores. Table row r = node;
  pair-row i = nodes (2i, 2i+1) packed as 128 bf16 = 256 B, so pair indices
  fit int16 (max 25087 < 32767) and one InstDMAGatherAnt fetches thousands
  of rows per instruction (vs one 128-row indirect DMA per slot column).
- Blocks are grouped into chunks of G=4; per-chunk slot capacity Cc = max
  in-degree in the chunk's rank groups. Slots gather the PAIR containing the
  src node; a static half-mask kills the wrong half and pad slots.
- Device per layer:
    node pass:  per block one matmul r=[h@(W Q) | h@(W a_dst)] -> radj (bf16)
                kept in SBUF (self-loop contributions read locally) and
                DMA'd row-major to tabA.
    exchange:   AllGather (bf16) -> Shared tabFull [NPAD, 64].
    edge pass:  per chunk: dma_gather pair rows -> [128, G*Cc, 128] bf16;
                w = max(exp(e), exp(0.2 e)) (Exp-only scalar table; no Lrelu
                table thrash), masked; unnormalized weighted sums via
                contiguous pairwise tree reduction (no strided reduce);
                add self term, normalize by the accumulated denominator,
                transpose+reconstruct through Q per 4-block PE group with
                fused bias+relu on DVE.
- Pooling: segment-max via dma_gather from local h3loc (sentinel -3e38),
  pairwise max tree, AllReduce(max), then fc + log_softmax on every core.
"""
import sys

sys.path.insert(0, "/opt/trn_rl_repo")

import numpy as np

import concourse.bass as bass
import concourse.bacc as bacc
import concourse.tile as tile
import concourse.mybir as mybir
from concourse import bass_utils
from concourse.masks import make_identity

N_CORES = 8
D = 64
N_LAYERS = 3
N_CLASSES = 10
N_GRAPHS = 512
NEG_SLOPE = 0.2
P = 128
GBLK = 4                  # blocks per chunk
MAX_GATHER_COLS = 34      # small pieces spread chunks across queues

_COMPILED = {}


def _householder_first_col(a):
    """Orthogonal symmetric Q with Q @ e0 = a/||a||."""
    a = np.asarray(a, np.float64)
    ah = a / np.linalg.norm(a)
    e0 = np.zeros_like(ah)
    e0[0] = 1.0
    u = ah - e0
    nu = np.linalg.norm(u)
    if nu < 1e-12:
        return np.eye(len(a))
    u = u / nu
    return np.eye(len(a)) - 2.0 * np.outer(u, u)


def _idx16_of(V):
    """[128, S] int -> int16 idx tile [128, 8S] (16-part pattern replicated x8).

    Gather position i = col*128 + p reads idx[i%16, i//16]; with
    i//16 = col*8 + p//16 the transform is a reshape/transpose.
    """
    Pn, S = V.shape
    assert Pn == 128
    t = V.reshape(8, 16, S).transpose(1, 2, 0).reshape(16, 8 * S)
    return np.tile(t.astype(np.int16), (8, 1))


def _host_prep(x, edge_index, batch, Ws, a_src, a_dst, bs, fc_w, fc_b):
    N = x.shape[0]
    src0 = np.asarray(edge_index[0], np.int64)
    dst0 = np.asarray(edge_index[1], np.int64)
    batch = np.asarray(batch, np.int64)

    NBLK_TOT = -(-N // P)
    NBLK_TOT = ((NBLK_TOT + N_CORES - 1) // N_CORES) * N_CORES
    NPAD = NBLK_TOT * P
    NB = NBLK_TOT // N_CORES
    NPC = NB * P
    NPAIR = NPAD // 2

    # in-degree (excluding self loops; those are handled locally on-chip)
    deg = np.zeros(NPAD, np.int64)
    np.add.at(deg, dst0, 1)
    order = np.argsort(-deg, kind="stable")
    new_id = np.empty(NPAD, np.int64)
    new_id[order] = np.arange(NPAD)
    k = np.arange(NPAD) // P
    p_in_blk = np.arange(NPAD) % P
    row_of_pos = (k % N_CORES) * NPC + (k // N_CORES) * P + p_in_blk
    row_of_old = row_of_pos[new_id]

    src_r = row_of_old[src0]
    dst_r = row_of_old[dst0]

    # per (core, block, partition) in-edge counts -> rank-group capacities
    core_of = dst_r // NPC
    j_of = (dst_r % NPC) // P
    p_of = dst_r % P
    cnt = np.zeros((N_CORES, NB, P), np.int64)
    np.add.at(cnt, (core_of, j_of, p_of), 1)
    C = cnt.max(axis=(0, 2))  # [NB] shared across cores

    # chunks of up to GBLK consecutive blocks, padded to chunk max
    chunks = []  # (j0, g, Cc, ocol)
    ocol = 0
    j0 = 0
    while j0 < NB:
        g = min(GBLK, NB - j0)
        Cc = int(C[j0:j0 + g].max())
        Cc = max(Cc, 1)
        chunks.append((j0, g, Cc, ocol))
        ocol += g * Cc
        j0 += g
    S = ocol

    # slot fill (vectorized): rank of each edge within its dst
    eorder = np.argsort(dst_r, kind="stable")
    dst_s = dst_r[eorder]
    src_s = src_r[eorder]
    dcount = np.zeros(NPAD + 1, np.int64)
    np.add.at(dcount, dst_s, 1)
    starts = np.zeros(NPAD + 1, np.int64)
    np.cumsum(dcount[:-1], out=starts[1:])
    rank_e = np.arange(len(dst_s)) - starts[dst_s]

    ci_of_j = np.zeros(NB, np.int64)
    colbase_of_j = np.zeros(NB, np.int64)
    Cc_of_j = np.zeros(NB, np.int64)
    for ci, (j0, g, Cc, oc) in enumerate(chunks):
        for gg in range(g):
            ci_of_j[j0 + gg] = ci
            colbase_of_j[j0 + gg] = oc + gg * Cc
            Cc_of_j[j0 + gg] = Cc

    ecore = dst_s // NPC
    ej = (dst_s % NPC) // P
    ep = dst_s % P
    ecol = colbase_of_j[ej] + rank_e

    slotpair = np.zeros((N_CORES, P, S), np.int64)
    halfmask = np.zeros((N_CORES, P, 2 * S), np.float32)
    slotpair[ecore, ep, ecol] = src_s // 2
    halfmask[ecore, ep, 2 * ecol + (src_s % 2)] = 1.0

    # pooling: local rows grouped by graph (graph block q = graphs 128q..)
    GB = N_GRAPHS // P
    graph_of_row = np.full(NPAD, -1, np.int64)
    graph_of_row[row_of_old[:N]] = batch
    # local row = j*P + p on the owning core (independent of AG grouping)
    rr = np.arange(NPAD)
    real = graph_of_row >= 0
    # recover (core, local row) from table row via the inverse permutation
    pos_of_row = np.empty(NPAD, np.int64)
    pos_of_row[row_of_pos] = np.arange(NPAD)
    pos_r = pos_of_row[rr[real]]
    prc = (pos_r // P) % N_CORES
    ploc = (pos_r // (P * N_CORES)) * P + pos_r % P
    pg0 = graph_of_row[real]
    # balanced dealing: sort graphs by their max per-core count, deal
    # round-robin to the GB blocks; host un-permutes the output rows.
    cnt2 = np.zeros((N_CORES, N_GRAPHS), np.int64)
    np.add.at(cnt2, (prc, pg0), 1)
    mx = cnt2.max(axis=0)
    gorder = np.argsort(-mx, kind="stable")
    slot_of_graph = np.empty(N_GRAPHS, np.int64)
    slot_of_graph[gorder] = (np.arange(N_GRAPHS) % GB) * P + np.arange(N_GRAPHS) // GB
    pg = slot_of_graph[pg0]
    pool_cnt = np.zeros((N_CORES, GB, P), np.int64)
    np.add.at(pool_cnt, (prc, pg // P, pg % P), 1)
    PC = np.maximum(pool_cnt.max(axis=(0, 2)), 1)  # [GB]
    poffs = np.zeros(GB + 1, np.int64)
    np.cumsum(PC, out=poffs[1:])
    SG = int(poffs[-1])
    LSENT = NPC
    pool_slot = np.full((N_CORES, P, SG), LSENT, np.int64)
    # rank of local row within its (core, graph-slot)
    lcore = prc
    lloc = ploc
    pkey = lcore * N_GRAPHS + pg
    porder = np.argsort(pkey, kind="stable")
    pk_s = pkey[porder]
    lloc_s = lloc[porder]
    pstart = np.zeros(N_CORES * N_GRAPHS + 1, np.int64)
    pc2 = np.zeros(N_CORES * N_GRAPHS + 1, np.int64)
    np.add.at(pc2, pk_s, 1)
    np.cumsum(pc2[:-1], out=pstart[1:])
    prank = np.arange(len(pk_s)) - pstart[pk_s]
    pcore_s = pk_s // N_GRAPHS
    pgr_s = pk_s % N_GRAPHS
    pool_slot[pcore_s, pgr_s % P, poffs[pgr_s // P] + prank] = lloc_s

    # weights
    Ws = np.asarray(Ws, np.float64)
    a_src = np.asarray(a_src, np.float64)
    a_dst = np.asarray(a_dst, np.float64)
    bs = np.asarray(bs, np.float64)
    NR = np.zeros((N_LAYERS, D, D + 1), np.float64)
    Qs = np.zeros((N_LAYERS, D, D), np.float64)
    s_l = np.zeros(N_LAYERS)
    for l in range(N_LAYERS):
        Q = _householder_first_col(a_src[l])
        Qs[l] = Q
        s_l[l] = np.linalg.norm(a_src[l])
        NR[l, :, :D] = Ws[l] @ Q
        NR[l, :, D] = Ws[l] @ a_dst[l]

    xpad = np.zeros((NPAD, D), np.float32)
    xpad[row_of_old[:N]] = np.asarray(x, np.float32)

    import ml_dtypes
    bf = ml_dtypes.bfloat16

    host = dict(
        NPAD=NPAD, NB=NB, NPC=NPC, NPAIR=NPAIR, S=S, SG=SG, GB=GB,
        chunks=chunks, PC=PC.astype(int), poffs=poffs.astype(int),
        s_l=s_l, slot_of_graph=slot_of_graph,
    )
    per_core = []
    for c in range(N_CORES):
        per_core.append({
            "xT": np.ascontiguousarray(
                xpad[c * NPC:(c + 1) * NPC].T.astype(bf)),
            "idx": np.ascontiguousarray(_idx16_of(slotpair[c])),
            "pidx": np.ascontiguousarray(_idx16_of(pool_slot[c])),
            "hmask": np.ascontiguousarray(halfmask[c].astype(bf)),
            "NR": np.ascontiguousarray(
                NR.transpose(1, 0, 2).reshape(D, N_LAYERS * (D + 1)).astype(bf)),
            "Qs": np.ascontiguousarray(
                Qs.transpose(1, 0, 2).reshape(D, N_LAYERS * D).astype(np.float32)),
            "bcol": np.ascontiguousarray(bs.T.astype(np.float32)),
            "brow3": np.ascontiguousarray(
                np.tile(bs[2][None, :].astype(np.float32), (P, 1))),
            "fcwT": np.ascontiguousarray(np.asarray(fc_w, np.float32).T),
            "fcb": np.ascontiguousarray(
                np.tile(np.asarray(fc_b, np.float32)[None, :], (P, 1))),
        })
    return host, per_core


def _build(host):
    NB, NPC, S, SG, GB = host["NB"], host["NPC"], host["S"], host["SG"], host["GB"]
    NPAD, NPAIR = host["NPAD"], host["NPAIR"]
    chunks = host["chunks"]
    PC, poffs = host["PC"], host["poffs"]
    s_l = host["s_l"]
    f32 = mybir.dt.float32
    bf16 = mybir.dt.bfloat16
    i16 = mybir.dt.int16
    AF = mybir.ActivationFunctionType
    OP = mybir.AluOpType

    nc = bacc.Bacc("TRN2", target_bir_lowering=False, debug=False, num_devices=N_CORES,
                   num_swdge_queues=4)
    t_xT = nc.dram_tensor("xT", [D, NPC], bf16, kind="ExternalInput")
    t_idx = nc.dram_tensor("idx", [P, 8 * S], i16, kind="ExternalInput")
    t_pidx = nc.dram_tensor("pidx", [P, 8 * SG], i16, kind="ExternalInput")
    t_hmask = nc.dram_tensor("hmask", [P, 2 * S], bf16, kind="ExternalInput")
    t_NR = nc.dram_tensor("NR", [D, N_LAYERS * (D + 1)], bf16, kind="ExternalInput")
    t_Qs = nc.dram_tensor("Qs", [D, N_LAYERS * D], f32, kind="ExternalInput")
    t_bcol = nc.dram_tensor("bcol", [D, N_LAYERS], f32, kind="ExternalInput")
    t_brow3 = nc.dram_tensor("brow3", [P, D], f32, kind="ExternalInput")
    t_fcwT = nc.dram_tensor("fcwT", [D, N_CLASSES], f32, kind="ExternalInput")
    t_fcb = nc.dram_tensor("fcb", [P, N_CLASSES], f32, kind="ExternalInput")
    t_out = nc.dram_tensor("out", [N_GRAPHS, N_CLASSES], f32, kind="ExternalOutput")

    t_tabA = nc.dram_tensor("tabA", [NPC, D], bf16)
    t_tabFull = nc.dram_tensor("tabFull", [NPAD, D], bf16, addr_space="Shared")
    t_h3loc = nc.dram_tensor("h3loc", [NPC + 1, D], f32)
    t_gpart = nc.dram_tensor("gpart", [N_GRAPHS, D], f32)
    t_gall = nc.dram_tensor("gall", [N_GRAPHS, D], f32)

    with tile.TileContext(nc) as tc:
        with (
            tc.tile_pool(name="persist", bufs=1) as pp,
            tc.tile_pool(name="gt", bufs=3) as gp,
            tc.tile_pool(name="pgp", bufs=2) as pgp,
            tc.tile_pool(name="work", bufs=3) as wp,
            tc.tile_pool(name="psA", bufs=2, space="PSUM") as psA,
            tc.tile_pool(name="psB", bufs=1, space="PSUM") as psB,
            tc.tile_pool(name="psC", bufs=2, space="PSUM") as psC,
        ):
            # persistent SBUF
            hT = pp.tile([D, NPC], bf16)
            radj = pp.tile([P, NB * (D + 1)], bf16)
            idxs = pp.tile([P, 8 * S], i16)
            pidx = pp.tile([P, 8 * SG], i16)
            hmask = pp.tile([P, 2 * S], bf16)
            NRt = pp.tile([D, N_LAYERS * (D + 1)], bf16)
            Qst = pp.tile([D, N_LAYERS * D], f32)
            bcol = pp.tile([D, N_LAYERS], f32)
            brow3 = pp.tile([P, D], f32)
            fcwT = pp.tile([D, N_CLASSES], f32)
            fcb = pp.tile([P, N_CLASSES], f32)
            ident = pp.tile([P, P], f32)
            sentP = pp.tile([1, D], f32)
            nc.sync.dma_start(hT[:], t_xT[:])
            nc.sync.dma_start(idxs[:], t_idx[:])
            nc.sync.dma_start(pidx[:], t_pidx[:])
            nc.sync.dma_start(hmask[:], t_hmask[:])
            nc.sync.dma_start(NRt[:], t_NR[:])
            nc.sync.dma_start(Qst[:], t_Qs[:])
            nc.sync.dma_start(bcol[:], t_bcol[:])
            nc.sync.dma_start(brow3[:], t_brow3[:])
            nc.sync.dma_start(fcwT[:], t_fcwT[:])
            nc.sync.dma_start(fcb[:], t_fcb[:])
            make_identity(nc, ident[:])
            nc.vector.memset(sentP[:], -3.0e38)
            nc.sync.dma_start(t_h3loc[NPC:NPC + 1, :], sentP[:])

            tab_pairs = t_tabFull[:].rearrange("(a b) d -> a (b d)", b=2)
            gq = [0]

            for l in range(N_LAYERS):
                sl = float(s_l[l])
                # ---- node pass ----
                for j in range(NB):
                    np_ps = psA.tile([P, D + 1], f32, tag="npp")
                    nc.tensor.matmul(
                        out=np_ps[:],
                        lhsT=hT[:, j * P:(j + 1) * P],
                        rhs=NRt[:, l * (D + 1):(l + 1) * (D + 1)],
                        start=True, stop=True,
                    )
                    nc.scalar.activation(
                        out=radj[:, j * (D + 1):(j + 1) * (D + 1)], in_=np_ps[:],
                        func=AF.Copy)
                    nc.sync.dma_start(
                        t_tabA[j * P:(j + 1) * P, :],
                        radj[:, j * (D + 1):j * (D + 1) + D])
                # ---- exchange ----
                nc.gpsimd.collective_compute(
                    "AllGather", mybir.AluOpType.bypass,
                    replica_groups=[list(range(N_CORES))],
                    ins=[t_tabA[:].opt()],
                    outs=[t_tabFull[:].opt()],
                )
                # ---- self-loop weights for all blocks: w = max(e^e2, e^.2e2)
                r0s = radj[:].rearrange("p (j e) -> p j e", e=D + 1)[:, :, 0:1] \
                    .rearrange("p j one -> p (j one)")
                ads = radj[:].rearrange("p (j e) -> p j e", e=D + 1)[:, :, D:D + 1] \
                    .rearrange("p j one -> p (j one)")
                adc = wp.tile([P, NB], f32, tag="adc")
                nc.scalar.activation(out=adc[:], in_=ads, func=AF.Copy)
                e2s = wp.tile([P, NB], f32, tag="e2s")
                nc.scalar.activation(out=e2s[:], in_=r0s, func=AF.Copy, scale=sl)
                nc.vector.tensor_tensor(out=e2s[:], in0=e2s[:], in1=adc[:], op=OP.add)
                wsA = wp.tile([P, NB], f32, tag="wsA")
                wself = wp.tile([P, NB], f32, tag="wself")
                nc.scalar.activation(out=wsA[:], in_=e2s[:], func=AF.Exp)
                nc.scalar.activation(out=wself[:], in_=e2s[:], func=AF.Exp,
                                     scale=NEG_SLOPE)
                nc.vector.tensor_tensor(out=wself[:], in0=wself[:], in1=wsA[:],
                                        op=OP.max)
                # ---- edge pass per chunk ----
                for (j0, g, Cc, oc) in chunks:
                    ncols = g * Cc
                    gt = gp.tile([P, ncols * P], bf16, tag="gt")
                    gt_g = gt[:].rearrange("p (k e) -> p k e", e=P)
                    npieces = -(-ncols // MAX_GATHER_COLS)
                    a = 0
                    for pi in range(npieces):
                        b = a + (ncols - a) // (npieces - pi)
                        nidx = (b - a) * P
                        nc.gpsimd.dma_gather(
                            out_ap=gt_g[:, a:b, :],
                            in_ap=tab_pairs,
                            idxs_ap=idxs[:, 8 * (oc + a):8 * (oc + b)],
                            num_idxs=nidx,
                            num_idxs_reg=nidx,
                            elem_size=P,
                            single_packet=False,
                            queue_num=gq[0] % 4,
                        )
                        gq[0] += 1
                        a = b
                    # attention weights over [P, 2*ncols]
                    r0 = gt[:].rearrange("p (k e) -> p k e", e=D)[:, :, 0:1] \
                        .rearrange("p k one -> p (k one)")
                    e2 = wp.tile([P, 2 * ncols], f32, tag="e2")
                    nc.scalar.activation(out=e2[:], in_=r0, func=AF.Copy, scale=sl)
                    nc.vector.tensor_tensor(
                        out=e2[:].rearrange("p (g c) -> p g c", g=g),
                        in0=e2[:].rearrange("p (g c) -> p g c", g=g),
                        in1=adc[:, j0:j0 + g].to_broadcast([P, g, 2 * Cc]),
                        op=OP.add)
                    wA = wp.tile([P, 2 * ncols], f32, tag="wA")
                    wB = wp.tile([P, 2 * ncols], f32, tag="wB")
                    nc.scalar.activation(out=wA[:], in_=e2[:], func=AF.Exp)
                    nc.scalar.activation(out=wB[:], in_=e2[:], func=AF.Exp,
                                         scale=NEG_SLOPE)
                    wmb = wp.tile([P, 2 * ncols], bf16, tag="wmb")
                    nc.vector.tensor_tensor(out=wmb[:], in0=wB[:], in1=wA[:],
                                            op=OP.max)
                    wm = wp.tile([P, 2 * ncols], bf16, tag="wm")
                    nc.vector.tensor_tensor(
                        out=wm[:], in0=wmb[:],
                        in1=hmask[:, 2 * oc:2 * (oc + ncols)], op=OP.mult)
                    # denominator per block
                    dn = wp.tile([P, g], f32, tag="dn")
                    nc.vector.reduce_sum(
                        out=dn[:],
                        in_=wm[:].rearrange("p (g c) -> p g c", g=g),
                        axis=mybir.AxisListType.X)
                    nc.vector.tensor_tensor(
                        out=dn[:], in0=dn[:], in1=wself[:, j0:j0 + g], op=OP.add)
                    recip = wp.tile([P, g], f32, tag="recip")
                    nc.vector.reciprocal(out=recip[:], in_=dn[:])
                    # weight the gathered rows (in place, pure bf16)
                    nc.vector.tensor_tensor(
                        out=gt[:].rearrange("p (k e) -> p k e", e=D),
                        in0=gt[:].rearrange("p (k e) -> p k e", e=D),
                        in1=wm[:].to_broadcast([P, 2 * ncols, D]),
                        op=OP.mult)
                    # in-place pairwise tree-sum over the 2*Cc slot axis
                    gt4 = gt[:].rearrange("p (g c d) -> p g c d", g=g, d=D)
                    cw = 2 * Cc
                    while cw > 1:
                        half = cw // 2
                        rem = cw - 2 * half
                        nc.vector.tensor_tensor(
                            out=gt4[:, :, 0:half, :],
                            in0=gt4[:, :, 0:half, :],
                            in1=gt4[:, :, half + rem:cw, :], op=OP.add)
                        cw = half + rem
                    # U = tree + self, normalized
                    selfr = radj[:].rearrange("p (j e) -> p j e", e=D + 1)[
                        :, j0:j0 + g, 0:D]
                    U = wp.tile([P, g * D], f32, tag="U")
                    nc.vector.tensor_tensor(
                        out=U[:].rearrange("p (g d) -> p g d", d=D),
                        in0=selfr,
                        in1=wself[:, j0:j0 + g].to_broadcast([P, g, D]),
                        op=OP.mult)
                    nc.vector.tensor_tensor(
                        out=U[:].rearrange("p (g d) -> p g d", d=D),
                        in0=U[:].rearrange("p (g d) -> p g d", d=D),
                        in1=gt4[:, :, 0:1, :].rearrange("p g one d -> p (g one) d"),
                        op=OP.add)
                    nc.vector.tensor_tensor(
                        out=U[:].rearrange("p (g d) -> p g d", d=D),
                        in0=U[:].rearrange("p (g d) -> p g d", d=D),
                        in1=recip[:].to_broadcast([P, g, D]),
                        op=OP.mult)
                    # transpose blocks into one PSUM tile -> aggT [64, g*128]
                    at_ps = psB.tile([D, g * P], f32, tag="at")
                    for gg in range(g):
                        nc.tensor.transpose(
                            out=at_ps[:, gg * P:(gg + 1) * P],
                            in_=U[:, gg * D:(gg + 1) * D],
                            identity=ident[:])
                    aggT = wp.tile([D, g * P], f32, tag="aggT")
                    nc.scalar.activation(out=aggT[:], in_=at_ps[:], func=AF.Copy)
                    if l < N_LAYERS - 1:
                        h_ps = psB.tile([D, g * P], f32, tag="hps")
                        nc.tensor.matmul(
                            out=h_ps[:], lhsT=Qst[:, l * D:(l + 1) * D],
                            rhs=aggT[:], start=True, stop=True)
                        nc.vector.tensor_scalar(
                            out=hT[:, j0 * P:(j0 + g) * P], in0=h_ps[:],
                            scalar1=bcol[:, l:l + 1], scalar2=0.0,
                            op0=OP.add, op1=OP.max)
                    else:
                        for gg in range(g):
                            h3_ps = psC.tile([P, D], f32, tag="h3ps")
                            nc.tensor.matmul(
                                out=h3_ps[:],
                                lhsT=aggT[:, gg * P:(gg + 1) * P],
                                rhs=Qst[:, l * D:(l + 1) * D],
                                start=True, stop=True)
                            h3 = wp.tile([P, D], f32, tag="h3")
                            nc.vector.tensor_tensor(
                                out=h3[:], in0=h3_ps[:], in1=brow3[:], op=OP.add)
                            nc.sync.dma_start(
                                t_h3loc[(j0 + gg) * P:(j0 + gg + 1) * P, :], h3[:])
            # ---- pooling: segment max over graphs ----
            for q in range(GB):
                PCq = int(PC[q])
                pg_t = pgp.tile([P, PCq * D], f32, tag="pg")
                pg_g = pg_t[:].rearrange("p (k e) -> p k e", e=D)
                a = 0
                while a < PCq:
                    b = min(a + MAX_GATHER_COLS, PCq)
                    nidx = (b - a) * P
                    nc.gpsimd.dma_gather(
                        out_ap=pg_g[:, a:b, :],
                        in_ap=t_h3loc[:],
                        idxs_ap=pidx[:, 8 * (poffs[q] + a):8 * (poffs[q] + b)],
                        num_idxs=nidx,
                        num_idxs_reg=nidx,
                        elem_size=D,
                        single_packet=False,
                        queue_num=gq[0] % 4,
                    )
                    gq[0] += 1
                    a = b
                cw = PCq
                while cw > 1:
                    half = cw // 2
                    rem = cw - 2 * half
                    nc.vector.tensor_tensor(
                        out=pg_g[:, 0:half, :], in0=pg_g[:, 0:half, :],
                        in1=pg_g[:, half + rem:cw, :], op=OP.max)
                    cw = half + rem
                nc.sync.dma_start(t_gpart[q * P:(q + 1) * P, :], pg_t[:, 0:D])
            nc.gpsimd.collective_compute(
                "AllReduce", mybir.AluOpType.max,
                replica_groups=[list(range(N_CORES))],
                ins=[t_gpart[:].opt()],
                outs=[t_gall[:].opt()],
            )
            # ---- fc + log_softmax (redundant on all cores) ----
            for q in range(GB):
                gsb = wp.tile([P, D], f32, tag="gsb")
                nc.sync.dma_start(gsb[:], t_gall[q * P:(q + 1) * P, :])
                mask = wp.tile([P, D], f32, tag="mask")
                nc.vector.tensor_scalar(
                    out=mask[:], in0=gsb[:], scalar1=-1.0e37, scalar2=None,
                    op0=OP.is_gt)
                nc.vector.tensor_tensor(out=gsb[:], in0=gsb[:], in1=mask[:],
                                        op=OP.mult)
                gT_ps = psB.tile([D, P], f32, tag="gT")
                nc.tensor.transpose(out=gT_ps[:], in_=gsb[:], identity=ident[:])
                gT = wp.tile([D, P], f32, tag="gTs")
                nc.vector.tensor_copy(out=gT[:], in_=gT_ps[:])
                lg_ps = psB.tile([P, N_CLASSES], f32, tag="lg")
                nc.tensor.matmul(out=lg_ps[:], lhsT=gT[:], rhs=fcwT[:],
                                 start=True, stop=True)
                lg = wp.tile([P, N_CLASSES], f32, tag="lgs")
                nc.vector.tensor_tensor(
                    out=lg[:], in0=lg_ps[:], in1=fcb[:], op=OP.add)
                m = wp.tile([P, 1], f32, tag="m")
                nc.vector.reduce_max(out=m[:], in_=lg[:], axis=mybir.AxisListType.X)
                mneg = wp.tile([P, 1], f32, tag="mneg")
                nc.vector.tensor_scalar_mul(out=mneg[:], in0=m[:], scalar1=-1.0)
                ex = wp.tile([P, N_CLASSES], f32, tag="ex")
                sumex = wp.tile([P, 1], f32, tag="sumex")
                nc.scalar.activation(out=ex[:], in_=lg[:], func=AF.Exp,
                                     bias=mneg[:], accum_out=sumex[:])
                logz = wp.tile([P, 1], f32, tag="logz")
                nc.scalar.activation(out=logz[:], in_=sumex[:], func=AF.Ln)
                off = wp.tile([P, 1], f32, tag="off")
                nc.vector.tensor_add(out=off[:], in0=m[:], in1=logz[:])
                outsb = wp.tile([P, N_CLASSES], f32, tag="outsb")
                nc.vector.tensor_tensor(
                    out=outsb[:], in0=lg[:],
                    in1=off[:].to_broadcast([P, N_CLASSES]), op=OP.subtract)
                nc.sync.dma_start(t_out[q * P:(q + 1) * P, :], outsb[:])
    nc.compile()
    return nc


def kernel(**inputs):
    x = np.asarray(inputs["x"])
    key = (x.shape, inputs["edge_index"].shape)
    host, per_core = _host_prep(**inputs)
    if key not in _COMPILED:
        _COMPILED[key] = _build(host)
    nc = _COMPILED[key]
    in_maps = [per_core[c] for c in range(N_CORES)]
    import os
    trace = False
    if os.environ.get("KERNEL_TRACE") == "1":
        try:
            import types
            if "antenv.axon_hooks" not in sys.modules:
                import antenv
                from trn_agent_boot.trn_boot import _ntff_profile_via_ctypes
                mod = types.ModuleType("antenv.axon_hooks")
                _state = {"hook": _ntff_profile_via_ctypes("/opt/axon/libaxon_pjrt.so")}
                mod.set_axon_ntff_profile_hook = lambda h: _state.__setitem__("hook", h)
                mod.get_axon_ntff_profile_hook = lambda: _state["hook"]
                sys.modules["antenv.axon_hooks"] = mod
                antenv.axon_hooks = mod
            trace = True
        except Exception:
            trace = False
    res = bass_utils.run_bass_kernel_spmd(
        nc, in_maps, core_ids=list(range(N_CORES)), trace=trace)
    globals()['LAST_EXEC_NS'] = res.exec_time_ns
    raw = np.asarray(res.results[0]["out"], np.float32)
    return raw[host["slot_of_graph"]]


LAST_EXEC_NS = None


# revision 26
# speedup vs baseline: 1.3031x; 1.3031x over previous
"""Distributed GAT (3-layer, heads=1) Bass kernel for 8 TRN2 NeuronCores.

Strategy (dst-sharded, batched dma_gather over a bf16 pair-row table):
- Host: permute nodes by in-degree (excl. self-loop) into degree-homogeneous
  blocks of 128, deal blocks round-robin to 8 cores. Table row r = node;
  pair-row i = nodes (2i, 2i+1) packed as 128 bf16 = 256 B, so pair indices
  fit int16 (max 25087 < 32767) and one InstDMAGatherAnt fetches thousands
  of rows per instruction (vs one 128-row indirect DMA per slot column).
- Blocks are grouped into chunks of G=4; per-chunk slot capacity Cc = max
  in-degree in the chunk's rank groups. Slots gather the PAIR containing the
  src node; a static half-mask kills the wrong half and pad slots.
- Device per layer:
    node pass:  per block one matmul r=[h@(W Q) | h@(W a_dst)] -> radj (bf16)
                kept in SBUF (self-loop contributions read locally) and
                DMA'd row-major to tabA.
    exchange:   AllGather (bf16) -> Shared tabFull [NPAD, 64].
    edge pass:  per chunk: dma_gather pair rows -> [128, G*Cc, 128] bf16;
                w = max(exp(e), exp(0.2 e)) (Exp-only scalar table; no Lrelu
                table thrash), masked; unnormalized weighted sums via
                contiguous pairwise tree reduction (no strided reduce);
                add self term, normalize by the accumulated denominator,
                transpose+reconstruct through Q per 4-block PE group with
                fused bias+relu on DVE.
- Pooling: segment-max via dma_gather from local h3loc (sentinel -3e38),
  pairwise max tree, AllReduce(max), then fc + log_softmax on every core.
"""
import sys

sys.path.insert(0, "/opt/trn_rl_repo")

import numpy as np

import concourse.bass as bass
import concourse.bacc as bacc
import concourse.tile as tile
import concourse.mybir as mybir
from concourse import bass_utils
from concourse.masks import make_identity

N_CORES = 8
D = 64
N_LAYERS = 3
N_CLASSES = 10
N_GRAPHS = 512
NEG_SLOPE = 0.2
P = 128
GBLK = 3                  # blocks per chunk
MAX_GATHER_COLS = 34      # small pieces spread chunks across queues

_COMPILED = {}


def _householder_first_col(a):
    """Orthogonal symmetric Q with Q @ e0 = a/||a||."""
    a = np.asarray(a, np.float64)
    ah = a / np.linalg.norm(a)
    e0 = np.zeros_like(ah)
    e0[0] = 1.0
    u = ah - e0
    nu = np.linalg.norm(u)
    if nu < 1e-12:
        return np.eye(len(a))
    u = u / nu
    return np.eye(len(a)) - 2.0 * np.outer(u, u)


def _idx16_of(V):
    """[128, S] int -> int16 idx tile [128, 8S] (16-part pattern replicated x8).

    Gather position i = col*128 + p reads idx[i%16, i//16]; with
    i//16 = col*8 + p//16 the transform is a reshape/transpose.
    """
    Pn, S = V.shape
    assert Pn == 128
    t = V.reshape(8, 16, S).transpose(1, 2, 0).reshape(16, 8 * S)
    return np.tile(t.astype(np.int16), (8, 1))


def _host_prep(x, edge_index, batch, Ws, a_src, a_dst, bs, fc_w, fc_b):
    N = x.shape[0]
    src0 = np.asarray(edge_index[0], np.int64)
    dst0 = np.asarray(edge_index[1], np.int64)
    batch = np.asarray(batch, np.int64)

    NBLK_TOT = -(-N // P)
    NBLK_TOT = ((NBLK_TOT + N_CORES - 1) // N_CORES) * N_CORES
    NPAD = NBLK_TOT * P
    NB = NBLK_TOT // N_CORES
    NPC = NB * P
    NPAIR = NPAD // 2

    # in-degree (excluding self loops; those are handled locally on-chip)
    deg = np.zeros(NPAD, np.int64)
    np.add.at(deg, dst0, 1)
    order = np.argsort(-deg, kind="stable")
    new_id = np.empty(NPAD, np.int64)
    new_id[order] = np.arange(NPAD)
    k = np.arange(NPAD) // P
    p_in_blk = np.arange(NPAD) % P
    row_of_pos = (k % N_CORES) * NPC + (k // N_CORES) * P + p_in_blk
    row_of_old = row_of_pos[new_id]

    src_r = row_of_old[src0]
    dst_r = row_of_old[dst0]

    # per (core, block, partition) in-edge counts -> rank-group capacities
    core_of = dst_r // NPC
    j_of = (dst_r % NPC) // P
    p_of = dst_r % P
    cnt = np.zeros((N_CORES, NB, P), np.int64)
    np.add.at(cnt, (core_of, j_of, p_of), 1)
    C = cnt.max(axis=(0, 2))  # [NB] shared across cores

    # chunks of up to GBLK consecutive blocks, padded to chunk max
    chunks = []  # (j0, g, Cc, ocol)
    ocol = 0
    j0 = 0
    while j0 < NB:
        g = min(GBLK, NB - j0)
        Cc = int(C[j0:j0 + g].max())
        Cc = max(Cc, 1)
        chunks.append((j0, g, Cc, ocol))
        ocol += g * Cc
        j0 += g
    S = ocol

    # slot fill (vectorized): rank of each edge within its dst
    eorder = np.argsort(dst_r, kind="stable")
    dst_s = dst_r[eorder]
    src_s = src_r[eorder]
    dcount = np.zeros(NPAD + 1, np.int64)
    np.add.at(dcount, dst_s, 1)
    starts = np.zeros(NPAD + 1, np.int64)
    np.cumsum(dcount[:-1], out=starts[1:])
    rank_e = np.arange(len(dst_s)) - starts[dst_s]

    ci_of_j = np.zeros(NB, np.int64)
    colbase_of_j = np.zeros(NB, np.int64)
    Cc_of_j = np.zeros(NB, np.int64)
    for ci, (j0, g, Cc, oc) in enumerate(chunks):
        for gg in range(g):
            ci_of_j[j0 + gg] = ci
            colbase_of_j[j0 + gg] = oc + gg * Cc
            Cc_of_j[j0 + gg] = Cc

    ecore = dst_s // NPC
    ej = (dst_s % NPC) // P
    ep = dst_s % P
    ecol = colbase_of_j[ej] + rank_e

    slotpair = np.zeros((N_CORES, P, S), np.int64)
    halfmask = np.zeros((N_CORES, P, 2 * S), np.float32)
    slotpair[ecore, ep, ecol] = src_s // 2
    halfmask[ecore, ep, 2 * ecol + (src_s % 2)] = 1.0

    # pooling: local rows grouped by graph (graph block q = graphs 128q..)
    GB = N_GRAPHS // P
    graph_of_row = np.full(NPAD, -1, np.int64)
    graph_of_row[row_of_old[:N]] = batch
    # local row = j*P + p on the owning core (independent of AG grouping)
    rr = np.arange(NPAD)
    real = graph_of_row >= 0
    # recover (core, local row) from table row via the inverse permutation
    pos_of_row = np.empty(NPAD, np.int64)
    pos_of_row[row_of_pos] = np.arange(NPAD)
    pos_r = pos_of_row[rr[real]]
    prc = (pos_r // P) % N_CORES
    ploc = (pos_r // (P * N_CORES)) * P + pos_r % P
    pg0 = graph_of_row[real]
    # balanced dealing: sort graphs by their max per-core count, deal
    # round-robin to the GB blocks; host un-permutes the output rows.
    cnt2 = np.zeros((N_CORES, N_GRAPHS), np.int64)
    np.add.at(cnt2, (prc, pg0), 1)
    mx = cnt2.max(axis=0)
    gorder = np.argsort(-mx, kind="stable")
    slot_of_graph = np.empty(N_GRAPHS, np.int64)
    slot_of_graph[gorder] = (np.arange(N_GRAPHS) % GB) * P + np.arange(N_GRAPHS) // GB
    pg = slot_of_graph[pg0]
    pool_cnt = np.zeros((N_CORES, GB, P), np.int64)
    np.add.at(pool_cnt, (prc, pg // P, pg % P), 1)
    PC = np.maximum(pool_cnt.max(axis=(0, 2)), 1)  # [GB]
    poffs = np.zeros(GB + 1, np.int64)
    np.cumsum(PC, out=poffs[1:])
    SG = int(poffs[-1])
    LSENT = NPC
    pool_slot = np.full((N_CORES, P, SG), LSENT, np.int64)
    # rank of local row within its (core, graph-slot)
    lcore = prc
    lloc = ploc
    pkey = lcore * N_GRAPHS + pg
    porder = np.argsort(pkey, kind="stable")
    pk_s = pkey[porder]
    lloc_s = lloc[porder]
    pstart = np.zeros(N_CORES * N_GRAPHS + 1, np.int64)
    pc2 = np.zeros(N_CORES * N_GRAPHS + 1, np.int64)
    np.add.at(pc2, pk_s, 1)
    np.cumsum(pc2[:-1], out=pstart[1:])
    prank = np.arange(len(pk_s)) - pstart[pk_s]
    pcore_s = pk_s // N_GRAPHS
    pgr_s = pk_s % N_GRAPHS
    pool_slot[pcore_s, pgr_s % P, poffs[pgr_s // P] + prank] = lloc_s

    # weights
    Ws = np.asarray(Ws, np.float64)
    a_src = np.asarray(a_src, np.float64)
    a_dst = np.asarray(a_dst, np.float64)
    bs = np.asarray(bs, np.float64)
    NR = np.zeros((N_LAYERS, D, D + 1), np.float64)
    Qs = np.zeros((N_LAYERS, D, D), np.float64)
    s_l = np.zeros(N_LAYERS)
    for l in range(N_LAYERS):
        Q = _householder_first_col(a_src[l])
        Qs[l] = Q
        s_l[l] = np.linalg.norm(a_src[l])
        NR[l, :, :D] = Ws[l] @ Q
        NR[l, :, D] = Ws[l] @ a_dst[l]

    xpad = np.zeros((NPAD, D), np.float32)
    xpad[row_of_old[:N]] = np.asarray(x, np.float32)

    import ml_dtypes
    bf = ml_dtypes.bfloat16

    host = dict(
        NPAD=NPAD, NB=NB, NPC=NPC, NPAIR=NPAIR, S=S, SG=SG, GB=GB,
        chunks=chunks, PC=PC.astype(int), poffs=poffs.astype(int),
        s_l=s_l, slot_of_graph=slot_of_graph,
    )
    per_core = []
    for c in range(N_CORES):
        per_core.append({
            "xT": np.ascontiguousarray(
                xpad[c * NPC:(c + 1) * NPC].T.astype(bf)),
            "idx": np.ascontiguousarray(_idx16_of(slotpair[c])),
            "pidx": np.ascontiguousarray(_idx16_of(pool_slot[c])),
            "hmask": np.ascontiguousarray(halfmask[c].astype(bf)),
            "NR": np.ascontiguousarray(
                NR.transpose(1, 0, 2).reshape(D, N_LAYERS * (D + 1)).astype(bf)),
            "Qs": np.ascontiguousarray(
                Qs.transpose(1, 0, 2).reshape(D, N_LAYERS * D).astype(np.float32)),
            "bcol": np.ascontiguousarray(bs.T.astype(np.float32)),
            "brow3": np.ascontiguousarray(
                np.tile(bs[2][None, :].astype(np.float32), (P, 1))),
            "fcwT": np.ascontiguousarray(np.asarray(fc_w, np.float32).T),
            "fcb": np.ascontiguousarray(
                np.tile(np.asarray(fc_b, np.float32)[None, :], (P, 1))),
        })
    return host, per_core


def _build(host):
    NB, NPC, S, SG, GB = host["NB"], host["NPC"], host["S"], host["SG"], host["GB"]
    NPAD, NPAIR = host["NPAD"], host["NPAIR"]
    chunks = host["chunks"]
    PC, poffs = host["PC"], host["poffs"]
    s_l = host["s_l"]
    f32 = mybir.dt.float32
    bf16 = mybir.dt.bfloat16
    i16 = mybir.dt.int16
    AF = mybir.ActivationFunctionType
    OP = mybir.AluOpType

    nc = bacc.Bacc("TRN2", target_bir_lowering=False, debug=False, num_devices=N_CORES,
                   num_swdge_queues=4)
    t_xT = nc.dram_tensor("xT", [D, NPC], bf16, kind="ExternalInput")
    t_idx = nc.dram_tensor("idx", [P, 8 * S], i16, kind="ExternalInput")
    t_pidx = nc.dram_tensor("pidx", [P, 8 * SG], i16, kind="ExternalInput")
    t_hmask = nc.dram_tensor("hmask", [P, 2 * S], bf16, kind="ExternalInput")
    t_NR = nc.dram_tensor("NR", [D, N_LAYERS * (D + 1)], bf16, kind="ExternalInput")
    t_Qs = nc.dram_tensor("Qs", [D, N_LAYERS * D], f32, kind="ExternalInput")
    t_bcol = nc.dram_tensor("bcol", [D, N_LAYERS], f32, kind="ExternalInput")
    t_brow3 = nc.dram_tensor("brow3", [P, D], f32, kind="ExternalInput")
    t_fcwT = nc.dram_tensor("fcwT", [D, N_CLASSES], f32, kind="ExternalInput")
    t_fcb = nc.dram_tensor("fcb", [P, N_CLASSES], f32, kind="ExternalInput")
    t_out = nc.dram_tensor("out", [N_GRAPHS, N_CLASSES], f32, kind="ExternalOutput")

    t_tabA = nc.dram_tensor("tabA", [NPC, D], bf16)
    t_tabFull = nc.dram_tensor("tabFull", [NPAD, D], bf16, addr_space="Shared")
    t_h3loc = nc.dram_tensor("h3loc", [NPC + 1, D], f32)
    t_gpart = nc.dram_tensor("gpart", [N_GRAPHS, D], f32)
    t_gall = nc.dram_tensor("gall", [N_GRAPHS, D], f32)

    with tile.TileContext(nc) as tc:
        with (
            tc.tile_pool(name="persist", bufs=1) as pp,
            tc.tile_pool(name="gt", bufs=4) as gp,
            tc.tile_pool(name="pgp", bufs=2) as pgp,
            tc.tile_pool(name="work", bufs=3) as wp,
            tc.tile_pool(name="psA", bufs=2, space="PSUM") as psA,
            tc.tile_pool(name="psB", bufs=1, space="PSUM") as psB,
            tc.tile_pool(name="psC", bufs=2, space="PSUM") as psC,
        ):
            # persistent SBUF
            hT = pp.tile([D, NPC], bf16)
            radj = pp.tile([P, NB * (D + 1)], bf16)
            idxs = pp.tile([P, 8 * S], i16)
            pidx = pp.tile([P, 8 * SG], i16)
            hmask = pp.tile([P, 2 * S], bf16)
            NRt = pp.tile([D, N_LAYERS * (D + 1)], bf16)
            Qst = pp.tile([D, N_LAYERS * D], f32)
            bcol = pp.tile([D, N_LAYERS], f32)
            brow3 = pp.tile([P, D], f32)
            fcwT = pp.tile([D, N_CLASSES], f32)
            fcb = pp.tile([P, N_CLASSES], f32)
            ident = pp.tile([P, P], f32)
            sentP = pp.tile([1, D], f32)
            nc.sync.dma_start(hT[:], t_xT[:])
            nc.sync.dma_start(idxs[:], t_idx[:])
            nc.sync.dma_start(pidx[:], t_pidx[:])
            nc.sync.dma_start(hmask[:], t_hmask[:])
            nc.sync.dma_start(NRt[:], t_NR[:])
            nc.sync.dma_start(Qst[:], t_Qs[:])
            nc.sync.dma_start(bcol[:], t_bcol[:])
            nc.sync.dma_start(brow3[:], t_brow3[:])
            nc.sync.dma_start(fcwT[:], t_fcwT[:])
            nc.sync.dma_start(fcb[:], t_fcb[:])
            make_identity(nc, ident[:])
            nc.vector.memset(sentP[:], -3.0e38)
            nc.sync.dma_start(t_h3loc[NPC:NPC + 1, :], sentP[:])

            tab_pairs = t_tabFull[:].rearrange("(a b) d -> a (b d)", b=2)
            gq = [0]

            for l in range(N_LAYERS):
                sl = float(s_l[l])
                # ---- node pass ----
                for j in range(NB):
                    np_ps = psA.tile([P, D + 1], f32, tag="npp")
                    nc.tensor.matmul(
                        out=np_ps[:],
                        lhsT=hT[:, j * P:(j + 1) * P],
                        rhs=NRt[:, l * (D + 1):(l + 1) * (D + 1)],
                        start=True, stop=True,
                    )
                    nc.scalar.activation(
                        out=radj[:, j * (D + 1):(j + 1) * (D + 1)], in_=np_ps[:],
                        func=AF.Copy)
                    nc.sync.dma_start(
                        t_tabA[j * P:(j + 1) * P, :],
                        radj[:, j * (D + 1):j * (D + 1) + D])
                # ---- exchange ----
                nc.gpsimd.collective_compute(
                    "AllGather", mybir.AluOpType.bypass,
                    replica_groups=[list(range(N_CORES))],
                    ins=[t_tabA[:].opt()],
                    outs=[t_tabFull[:].opt()],
                )
                # ---- self-loop weights for all blocks: w = max(e^e2, e^.2e2)
                r0s = radj[:].rearrange("p (j e) -> p j e", e=D + 1)[:, :, 0:1] \
                    .rearrange("p j one -> p (j one)")
                ads = radj[:].rearrange("p (j e) -> p j e", e=D + 1)[:, :, D:D + 1] \
                    .rearrange("p j one -> p (j one)")
                adc = wp.tile([P, NB], f32, tag="adc")
                nc.scalar.activation(out=adc[:], in_=ads, func=AF.Copy)
                e2s = wp.tile([P, NB], f32, tag="e2s")
                nc.scalar.activation(out=e2s[:], in_=r0s, func=AF.Copy, scale=sl)
                nc.vector.tensor_tensor(out=e2s[:], in0=e2s[:], in1=adc[:], op=OP.add)
                wsA = wp.tile([P, NB], f32, tag="wsA")
                wself = wp.tile([P, NB], f32, tag="wself")
                nc.scalar.activation(out=wsA[:], in_=e2s[:], func=AF.Exp)
                nc.scalar.activation(out=wself[:], in_=e2s[:], func=AF.Exp,
                                     scale=NEG_SLOPE)
                nc.vector.tensor_tensor(out=wself[:], in0=wself[:], in1=wsA[:],
                                        op=OP.max)
                # ---- edge pass per chunk ----
                for (j0, g, Cc, oc) in chunks:
                    ncols = g * Cc
                    gt = gp.tile([P, ncols * P], bf16, tag="gt")
                    gt_g = gt[:].rearrange("p (k e) -> p k e", e=P)
                    npieces = -(-ncols // MAX_GATHER_COLS)
                    a = 0
                    for pi in range(npieces):
                        b = a + (ncols - a) // (npieces - pi)
                        nidx = (b - a) * P
                        nc.gpsimd.dma_gather(
                            out_ap=gt_g[:, a:b, :],
                            in_ap=tab_pairs,
                            idxs_ap=idxs[:, 8 * (oc + a):8 * (oc + b)],
                            num_idxs=nidx,
                            num_idxs_reg=nidx,
                            elem_size=P,
                            single_packet=False,
                            queue_num=gq[0] % 4,
                        )
                        gq[0] += 1
                        a = b
                    # attention weights over [P, 2*ncols]
                    r0 = gt[:].rearrange("p (k e) -> p k e", e=D)[:, :, 0:1] \
                        .rearrange("p k one -> p (k one)")
                    e2 = wp.tile([P, 2 * ncols], f32, tag="e2")
                    nc.scalar.activation(out=e2[:], in_=r0, func=AF.Copy, scale=sl)
                    nc.vector.tensor_tensor(
                        out=e2[:].rearrange("p (g c) -> p g c", g=g),
                        in0=e2[:].rearrange("p (g c) -> p g c", g=g),
                        in1=adc[:, j0:j0 + g].to_broadcast([P, g, 2 * Cc]),
                        op=OP.add)
                    wA = wp.tile([P, 2 * ncols], f32, tag="wA")
                    wB = wp.tile([P, 2 * ncols], f32, tag="wB")
                    nc.scalar.activation(out=wA[:], in_=e2[:], func=AF.Exp)
                    nc.scalar.activation(out=wB[:], in_=e2[:], func=AF.Exp,
                                         scale=NEG_SLOPE)
                    wmb = wp.tile([P, 2 * ncols], bf16, tag="wmb")
                    nc.vector.tensor_tensor(out=wmb[:], in0=wB[:], in1=wA[:],
                                            op=OP.max)
                    wm = wp.tile([P, 2 * ncols], bf16, tag="wm")
                    nc.vector.tensor_tensor(
                        out=wm[:], in0=wmb[:],
                        in1=hmask[:, 2 * oc:2 * (oc + ncols)], op=OP.mult)
                    # denominator per block
                    dn = wp.tile([P, g], f32, tag="dn")
                    nc.vector.reduce_sum(
                        out=dn[:],
                        in_=wm[:].rearrange("p (g c) -> p g c", g=g),
                        axis=mybir.AxisListType.X)
                    nc.vector.tensor_tensor(
                        out=dn[:], in0=dn[:], in1=wself[:, j0:j0 + g], op=OP.add)
                    recip = wp.tile([P, g], f32, tag="recip")
                    nc.vector.reciprocal(out=recip[:], in_=dn[:])
                    # weight the gathered rows (in place, pure bf16)
                    nc.vector.tensor_tensor(
                        out=gt[:].rearrange("p (k e) -> p k e", e=D),
                        in0=gt[:].rearrange("p (k e) -> p k e", e=D),
                        in1=wm[:].to_broadcast([P, 2 * ncols, D]),
                        op=OP.mult)
                    # in-place pairwise tree-sum over the 2*Cc slot axis
                    gt4 = gt[:].rearrange("p (g c d) -> p g c d", g=g, d=D)
                    cw = 2 * Cc
                    while cw > 1:
                        half = cw // 2
                        rem = cw - 2 * half
                        nc.vector.tensor_tensor(
                            out=gt4[:, :, 0:half, :],
                            in0=gt4[:, :, 0:half, :],
                            in1=gt4[:, :, half + rem:cw, :], op=OP.add)
                        cw = half + rem
                    # U = tree + self, normalized
                    selfr = radj[:].rearrange("p (j e) -> p j e", e=D + 1)[
                        :, j0:j0 + g, 0:D]
                    U = wp.tile([P, g * D], f32, tag="U")
                    nc.vector.tensor_tensor(
                        out=U[:].rearrange("p (g d) -> p g d", d=D),
                        in0=selfr,
                        in1=wself[:, j0:j0 + g].to_broadcast([P, g, D]),
                        op=OP.mult)
                    nc.vector.tensor_tensor(
                        out=U[:].rearrange("p (g d) -> p g d", d=D),
                        in0=U[:].rearrange("p (g d) -> p g d", d=D),
                        in1=gt4[:, :, 0:1, :].rearrange("p g one d -> p (g one) d"),
                        op=OP.add)
                    nc.vector.tensor_tensor(
                        out=U[:].rearrange("p (g d) -> p g d", d=D),
                        in0=U[:].rearrange("p (g d) -> p g d", d=D),
                        in1=recip[:].to_broadcast([P, g, D]),
                        op=OP.mult)
                    # transpose blocks into one PSUM tile -> aggT [64, g*128]
                    at_ps = psB.tile([D, g * P], f32, tag="at")
                    for gg in range(g):
                        nc.tensor.transpose(
                            out=at_ps[:, gg * P:(gg + 1) * P],
                            in_=U[:, gg * D:(gg + 1) * D],
                            identity=ident[:])
                    aggT = wp.tile([D, g * P], f32, tag="aggT")
                    nc.scalar.activation(out=aggT[:], in_=at_ps[:], func=AF.Copy)
                    if l < N_LAYERS - 1:
                        h_ps = psB.tile([D, g * P], f32, tag="hps")
                        nc.tensor.matmul(
                            out=h_ps[:], lhsT=Qst[:, l * D:(l + 1) * D],
                            rhs=aggT[:], start=True, stop=True)
                        nc.vector.tensor_scalar(
                            out=hT[:, j0 * P:(j0 + g) * P], in0=h_ps[:],
                            scalar1=bcol[:, l:l + 1], scalar2=0.0,
                            op0=OP.add, op1=OP.max)
                    else:
                        for gg in range(g):
                            h3_ps = psC.tile([P, D], f32, tag="h3ps")
                            nc.tensor.matmul(
                                out=h3_ps[:],
                                lhsT=aggT[:, gg * P:(gg + 1) * P],
                                rhs=Qst[:, l * D:(l + 1) * D],
                                start=True, stop=True)
                            h3 = wp.tile([P, D], f32, tag="h3")
                            nc.vector.tensor_tensor(
                                out=h3[:], in0=h3_ps[:], in1=brow3[:], op=OP.add)
                            nc.sync.dma_start(
                                t_h3loc[(j0 + gg) * P:(j0 + gg + 1) * P, :], h3[:])
            # ---- pooling: segment max over graphs ----
            for q in range(GB):
                PCq = int(PC[q])
                pg_t = pgp.tile([P, PCq * D], f32, tag="pg")
                pg_g = pg_t[:].rearrange("p (k e) -> p k e", e=D)
                a = 0
                while a < PCq:
                    b = min(a + MAX_GATHER_COLS, PCq)
                    nidx = (b - a) * P
                    nc.gpsimd.dma_gather(
                        out_ap=pg_g[:, a:b, :],
                        in_ap=t_h3loc[:],
                        idxs_ap=pidx[:, 8 * (poffs[q] + a):8 * (poffs[q] + b)],
                        num_idxs=nidx,
                        num_idxs_reg=nidx,
                        elem_size=D,
                        single_packet=False,
                        queue_num=gq[0] % 4,
                    )
                    gq[0] += 1
                    a = b
                cw = PCq
                while cw > 1:
                    half = cw // 2
                    rem = cw - 2 * half
                    nc.vector.tensor_tensor(
                        out=pg_g[:, 0:half, :], in0=pg_g[:, 0:half, :],
                        in1=pg_g[:, half + rem:cw, :], op=OP.max)
                    cw = half + rem
                nc.sync.dma_start(t_gpart[q * P:(q + 1) * P, :], pg_t[:, 0:D])
            nc.gpsimd.collective_compute(
                "AllReduce", mybir.AluOpType.max,
                replica_groups=[list(range(N_CORES))],
                ins=[t_gpart[:].opt()],
                outs=[t_gall[:].opt()],
            )
            # ---- fc + log_softmax (redundant on all cores) ----
            for q in range(GB):
                gsb = wp.tile([P, D], f32, tag="gsb")
                nc.sync.dma_start(gsb[:], t_gall[q * P:(q + 1) * P, :])
                mask = wp.tile([P, D], f32, tag="mask")
                nc.vector.tensor_scalar(
                    out=mask[:], in0=gsb[:], scalar1=-1.0e37, scalar2=None,
                    op0=OP.is_gt)
                nc.vector.tensor_tensor(out=gsb[:], in0=gsb[:], in1=mask[:],
                                        op=OP.mult)
                gT_ps = psB.tile([D, P], f32, tag="gT")
                nc.tensor.transpose(out=gT_ps[:], in_=gsb[:], identity=ident[:])
                gT = wp.tile([D, P], f32, tag="gTs")
                nc.vector.tensor_copy(out=gT[:], in_=gT_ps[:])
                lg_ps = psB.tile([P, N_CLASSES], f32, tag="lg")
                nc.tensor.matmul(out=lg_ps[:], lhsT=gT[:], rhs=fcwT[:],
                                 start=True, stop=True)
                lg = wp.tile([P, N_CLASSES], f32, tag="lgs")
                nc.vector.tensor_tensor(
                    out=lg[:], in0=lg_ps[:], in1=fcb[:], op=OP.add)
                m = wp.tile([P, 1], f32, tag="m")
                nc.vector.reduce_max(out=m[:], in_=lg[:], axis=mybir.AxisListType.X)
                mneg = wp.tile([P, 1], f32, tag="mneg")
                nc.vector.tensor_scalar_mul(out=mneg[:], in0=m[:], scalar1=-1.0)
                ex = wp.tile([P, N_CLASSES], f32, tag="ex")
                sumex = wp.tile([P, 1], f32, tag="sumex")
                nc.scalar.activation(out=ex[:], in_=lg[:], func=AF.Exp,
                                     bias=mneg[:], accum_out=sumex[:])
                logz = wp.tile([P, 1], f32, tag="logz")
                nc.scalar.activation(out=logz[:], in_=sumex[:], func=AF.Ln)
                off = wp.tile([P, 1], f32, tag="off")
                nc.vector.tensor_add(out=off[:], in0=m[:], in1=logz[:])
                outsb = wp.tile([P, N_CLASSES], f32, tag="outsb")
                nc.vector.tensor_tensor(
                    out=outsb[:], in0=lg[:],
                    in1=off[:].to_broadcast([P, N_CLASSES]), op=OP.subtract)
                nc.sync.dma_start(t_out[q * P:(q + 1) * P, :], outsb[:])
    nc.compile()
    return nc


def kernel(**inputs):
    x = np.asarray(inputs["x"])
    key = (x.shape, inputs["edge_index"].shape)
    host, per_core = _host_prep(**inputs)
    if key not in _COMPILED:
        _COMPILED[key] = _build(host)
    nc = _COMPILED[key]
    in_maps = [per_core[c] for c in range(N_CORES)]
    import os
    trace = False
    if os.environ.get("KERNEL_TRACE") == "1":
        try:
            import types
            if "antenv.axon_hooks" not in sys.modules:
                import antenv
                from trn_agent_boot.trn_boot import _ntff_profile_via_ctypes
                mod = types.ModuleType("antenv.axon_hooks")
                _state = {"hook": _ntff_profile_via_ctypes("/opt/axon/libaxon_pjrt.so")}
                mod.set_axon_ntff_profile_hook = lambda h: _state.__setitem__("hook", h)
                mod.get_axon_ntff_profile_hook = lambda: _state["hook"]
                sys.modules["antenv.axon_hooks"] = mod
                antenv.axon_hooks = mod
            trace = True
        except Exception:
            trace = False
    res = bass_utils.run_bass_kernel_spmd(
        nc, in_maps, core_ids=list(range(N_CORES)), trace=trace)
    globals()['LAST_EXEC_NS'] = res.exec_time_ns
    raw = np.asarray(res.results[0]["out"], np.float32)
    return raw[host["slot_of_graph"]]


LAST_EXEC_NS = None


# revision 27
# speedup vs baseline: 1.3344x; 1.0240x over previous
"""Distributed GAT (3-layer, heads=1) Bass kernel for 8 TRN2 NeuronCores.

Strategy (dst-sharded, batched dma_gather over a bf16 pair-row table):
- Host: permute nodes by in-degree (excl. self-loop) into degree-homogeneous
  blocks of 128, deal blocks round-robin to 8 cores. Table row r = node;
  pair-row i = nodes (2i, 2i+1) packed as 128 bf16 = 256 B, so pair indices
  fit int16 (max 25087 < 32767) and one InstDMAGatherAnt fetches thousands
  of rows per instruction (vs one 128-row indirect DMA per slot column).
- Blocks are grouped into chunks of G=4; per-chunk slot capacity Cc = max
  in-degree in the chunk's rank groups. Slots gather the PAIR containing the
  src node; a static half-mask kills the wrong half and pad slots.
- Device per layer:
    node pass:  per block one matmul r=[h@(W Q) | h@(W a_dst)] -> radj (bf16)
                kept in SBUF (self-loop contributions read locally) and
                DMA'd row-major to tabA.
    exchange:   AllGather (bf16) -> Shared tabFull [NPAD, 64].
    edge pass:  per chunk: dma_gather pair rows -> [128, G*Cc, 128] bf16;
                w = max(exp(e), exp(0.2 e)) (Exp-only scalar table; no Lrelu
                table thrash), masked; unnormalized weighted sums via
                contiguous pairwise tree reduction (no strided reduce);
                add self term, normalize by the accumulated denominator,
                transpose+reconstruct through Q per 4-block PE group with
                fused bias+relu on DVE.
- Pooling: segment-max via dma_gather from local h3loc (sentinel -3e38),
  pairwise max tree, AllReduce(max), then fc + log_softmax on every core.
"""
import sys

sys.path.insert(0, "/opt/trn_rl_repo")

import numpy as np

import concourse.bass as bass
import concourse.bacc as bacc
import concourse.tile as tile
import concourse.mybir as mybir
from concourse import bass_utils
from concourse.masks import make_identity

N_CORES = 8
D = 64
N_LAYERS = 3
N_CLASSES = 10
N_GRAPHS = 512
NEG_SLOPE = 0.2
P = 128
GBLK = 3                  # blocks per chunk
MAX_GATHER_COLS = 34      # small pieces spread chunks across queues

_COMPILED = {}


def _householder_first_col(a):
    """Orthogonal symmetric Q with Q @ e0 = a/||a||."""
    a = np.asarray(a, np.float64)
    ah = a / np.linalg.norm(a)
    e0 = np.zeros_like(ah)
    e0[0] = 1.0
    u = ah - e0
    nu = np.linalg.norm(u)
    if nu < 1e-12:
        return np.eye(len(a))
    u = u / nu
    return np.eye(len(a)) - 2.0 * np.outer(u, u)


def _idx16_of(V):
    """[128, S] int -> int16 idx tile [128, 8S] (16-part pattern replicated x8).

    Gather position i = col*128 + p reads idx[i%16, i//16]; with
    i//16 = col*8 + p//16 the transform is a reshape/transpose.
    """
    Pn, S = V.shape
    assert Pn == 128
    t = V.reshape(8, 16, S).transpose(1, 2, 0).reshape(16, 8 * S)
    return np.tile(t.astype(np.int16), (8, 1))


def _host_prep(x, edge_index, batch, Ws, a_src, a_dst, bs, fc_w, fc_b):
    N = x.shape[0]
    src0 = np.asarray(edge_index[0], np.int64)
    dst0 = np.asarray(edge_index[1], np.int64)
    batch = np.asarray(batch, np.int64)

    NBLK_TOT = -(-N // P)
    NBLK_TOT = ((NBLK_TOT + N_CORES - 1) // N_CORES) * N_CORES
    NPAD = NBLK_TOT * P
    NB = NBLK_TOT // N_CORES
    NPC = NB * P
    NPAIR = NPAD // 2

    # in-degree (excluding self loops; those are handled locally on-chip)
    deg = np.zeros(NPAD, np.int64)
    np.add.at(deg, dst0, 1)
    order = np.argsort(-deg, kind="stable")
    new_id = np.empty(NPAD, np.int64)
    new_id[order] = np.arange(NPAD)
    k = np.arange(NPAD) // P
    p_in_blk = np.arange(NPAD) % P
    row_of_pos = (k % N_CORES) * NPC + (k // N_CORES) * P + p_in_blk
    row_of_old = row_of_pos[new_id]

    src_r = row_of_old[src0]
    dst_r = row_of_old[dst0]

    # per (core, block, partition) in-edge counts -> rank-group capacities
    core_of = dst_r // NPC
    j_of = (dst_r % NPC) // P
    p_of = dst_r % P
    cnt = np.zeros((N_CORES, NB, P), np.int64)
    np.add.at(cnt, (core_of, j_of, p_of), 1)
    C = cnt.max(axis=(0, 2))  # [NB] shared across cores

    # chunks of up to GBLK consecutive blocks, padded to chunk max
    chunks = []  # (j0, g, Cc, ocol)
    ocol = 0
    j0 = 0
    while j0 < NB:
        g = min(GBLK, NB - j0)
        Cc = int(C[j0:j0 + g].max())
        Cc = max(Cc, 1)
        chunks.append((j0, g, Cc, ocol))
        ocol += g * Cc
        j0 += g
    S = ocol

    # slot fill (vectorized): rank of each edge within its dst
    eorder = np.argsort(dst_r, kind="stable")
    dst_s = dst_r[eorder]
    src_s = src_r[eorder]
    dcount = np.zeros(NPAD + 1, np.int64)
    np.add.at(dcount, dst_s, 1)
    starts = np.zeros(NPAD + 1, np.int64)
    np.cumsum(dcount[:-1], out=starts[1:])
    rank_e = np.arange(len(dst_s)) - starts[dst_s]

    ci_of_j = np.zeros(NB, np.int64)
    colbase_of_j = np.zeros(NB, np.int64)
    Cc_of_j = np.zeros(NB, np.int64)
    for ci, (j0, g, Cc, oc) in enumerate(chunks):
        for gg in range(g):
            ci_of_j[j0 + gg] = ci
            colbase_of_j[j0 + gg] = oc + gg * Cc
            Cc_of_j[j0 + gg] = Cc

    ecore = dst_s // NPC
    ej = (dst_s % NPC) // P
    ep = dst_s % P
    ecol = colbase_of_j[ej] + rank_e

    slotpair = np.zeros((N_CORES, P, S), np.int64)
    halfmask = np.zeros((N_CORES, P, 2 * S), np.float32)
    slotpair[ecore, ep, ecol] = src_s // 2
    halfmask[ecore, ep, 2 * ecol + (src_s % 2)] = 1.0

    # pooling: local rows grouped by graph (graph block q = graphs 128q..)
    GB = N_GRAPHS // P
    graph_of_row = np.full(NPAD, -1, np.int64)
    graph_of_row[row_of_old[:N]] = batch
    # local row = j*P + p on the owning core (independent of AG grouping)
    rr = np.arange(NPAD)
    real = graph_of_row >= 0
    # recover (core, local row) from table row via the inverse permutation
    pos_of_row = np.empty(NPAD, np.int64)
    pos_of_row[row_of_pos] = np.arange(NPAD)
    pos_r = pos_of_row[rr[real]]
    prc = (pos_r // P) % N_CORES
    ploc = (pos_r // (P * N_CORES)) * P + pos_r % P
    pg0 = graph_of_row[real]
    # balanced dealing: sort graphs by their max per-core count, deal
    # round-robin to the GB blocks; host un-permutes the output rows.
    cnt2 = np.zeros((N_CORES, N_GRAPHS), np.int64)
    np.add.at(cnt2, (prc, pg0), 1)
    mx = cnt2.max(axis=0)
    gorder = np.argsort(-mx, kind="stable")
    slot_of_graph = np.empty(N_GRAPHS, np.int64)
    slot_of_graph[gorder] = (np.arange(N_GRAPHS) % GB) * P + np.arange(N_GRAPHS) // GB
    pg = slot_of_graph[pg0]
    pool_cnt = np.zeros((N_CORES, GB, P), np.int64)
    np.add.at(pool_cnt, (prc, pg // P, pg % P), 1)
    PC = np.maximum(pool_cnt.max(axis=(0, 2)), 1)  # [GB]
    poffs = np.zeros(GB + 1, np.int64)
    np.cumsum(PC, out=poffs[1:])
    SG = int(poffs[-1])
    LSENT = NPC
    pool_slot = np.full((N_CORES, P, SG), LSENT, np.int64)
    # rank of local row within its (core, graph-slot)
    lcore = prc
    lloc = ploc
    pkey = lcore * N_GRAPHS + pg
    porder = np.argsort(pkey, kind="stable")
    pk_s = pkey[porder]
    lloc_s = lloc[porder]
    pstart = np.zeros(N_CORES * N_GRAPHS + 1, np.int64)
    pc2 = np.zeros(N_CORES * N_GRAPHS + 1, np.int64)
    np.add.at(pc2, pk_s, 1)
    np.cumsum(pc2[:-1], out=pstart[1:])
    prank = np.arange(len(pk_s)) - pstart[pk_s]
    pcore_s = pk_s // N_GRAPHS
    pgr_s = pk_s % N_GRAPHS
    pool_slot[pcore_s, pgr_s % P, poffs[pgr_s // P] + prank] = lloc_s

    # weights
    Ws = np.asarray(Ws, np.float64)
    a_src = np.asarray(a_src, np.float64)
    a_dst = np.asarray(a_dst, np.float64)
    bs = np.asarray(bs, np.float64)
    NR = np.zeros((N_LAYERS, D, D + 1), np.float64)
    Qs = np.zeros((N_LAYERS, D, D), np.float64)
    s_l = np.zeros(N_LAYERS)
    for l in range(N_LAYERS):
        Q = _householder_first_col(a_src[l])
        Qs[l] = Q
        s_l[l] = np.linalg.norm(a_src[l])
        NR[l, :, :D] = Ws[l] @ Q
        NR[l, :, D] = Ws[l] @ a_dst[l]

    xpad = np.zeros((NPAD, D), np.float32)
    xpad[row_of_old[:N]] = np.asarray(x, np.float32)

    import ml_dtypes
    bf = ml_dtypes.bfloat16

    host = dict(
        NPAD=NPAD, NB=NB, NPC=NPC, NPAIR=NPAIR, S=S, SG=SG, GB=GB,
        chunks=chunks, PC=PC.astype(int), poffs=poffs.astype(int),
        s_l=s_l, slot_of_graph=slot_of_graph,
    )
    per_core = []
    for c in range(N_CORES):
        per_core.append({
            "xT": np.ascontiguousarray(
                xpad[c * NPC:(c + 1) * NPC].T.astype(bf)),
            "idx": np.ascontiguousarray(_idx16_of(slotpair[c])),
            "pidx": np.ascontiguousarray(_idx16_of(pool_slot[c])),
            "hmask": np.ascontiguousarray(halfmask[c].astype(bf)),
            "NR": np.ascontiguousarray(
                NR.transpose(1, 0, 2).reshape(D, N_LAYERS * (D + 1)).astype(bf)),
            "Qs": np.ascontiguousarray(
                Qs.transpose(1, 0, 2).reshape(D, N_LAYERS * D).astype(np.float32)),
            "bcol": np.ascontiguousarray(bs.T.astype(np.float32)),
            "brow3": np.ascontiguousarray(
                np.tile(bs[2][None, :].astype(np.float32), (P, 1))),
            "fcwT": np.ascontiguousarray(np.asarray(fc_w, np.float32).T),
            "fcb": np.ascontiguousarray(
                np.tile(np.asarray(fc_b, np.float32)[None, :], (P, 1))),
        })
    return host, per_core


def _build(host):
    NB, NPC, S, SG, GB = host["NB"], host["NPC"], host["S"], host["SG"], host["GB"]
    NPAD, NPAIR = host["NPAD"], host["NPAIR"]
    chunks = host["chunks"]
    PC, poffs = host["PC"], host["poffs"]
    s_l = host["s_l"]
    f32 = mybir.dt.float32
    bf16 = mybir.dt.bfloat16
    i16 = mybir.dt.int16
    AF = mybir.ActivationFunctionType
    OP = mybir.AluOpType

    nc = bacc.Bacc("TRN2", target_bir_lowering=False, debug=False, num_devices=N_CORES,
                   num_swdge_queues=4)
    t_xT = nc.dram_tensor("xT", [D, NPC], bf16, kind="ExternalInput")
    t_idx = nc.dram_tensor("idx", [P, 8 * S], i16, kind="ExternalInput")
    t_pidx = nc.dram_tensor("pidx", [P, 8 * SG], i16, kind="ExternalInput")
    t_hmask = nc.dram_tensor("hmask", [P, 2 * S], bf16, kind="ExternalInput")
    t_NR = nc.dram_tensor("NR", [D, N_LAYERS * (D + 1)], bf16, kind="ExternalInput")
    t_Qs = nc.dram_tensor("Qs", [D, N_LAYERS * D], f32, kind="ExternalInput")
    t_bcol = nc.dram_tensor("bcol", [D, N_LAYERS], f32, kind="ExternalInput")
    t_brow3 = nc.dram_tensor("brow3", [P, D], f32, kind="ExternalInput")
    t_fcwT = nc.dram_tensor("fcwT", [D, N_CLASSES], f32, kind="ExternalInput")
    t_fcb = nc.dram_tensor("fcb", [P, N_CLASSES], f32, kind="ExternalInput")
    t_out = nc.dram_tensor("out", [N_GRAPHS, N_CLASSES], f32, kind="ExternalOutput")

    t_tabA = nc.dram_tensor("tabA", [NPC, D], bf16)
    t_tabFull = nc.dram_tensor("tabFull", [NPAD, D], bf16, addr_space="Shared")
    t_h3loc = nc.dram_tensor("h3loc", [NPC + 1, D], f32)
    t_gpart = nc.dram_tensor("gpart", [N_GRAPHS, D], f32)
    t_gall = nc.dram_tensor("gall", [N_GRAPHS, D], f32)

    with tile.TileContext(nc) as tc:
        with (
            tc.tile_pool(name="persist", bufs=1) as pp,
            tc.tile_pool(name="gt", bufs=5) as gp,
            tc.tile_pool(name="pgp", bufs=1) as pgp,
            tc.tile_pool(name="work", bufs=3) as wp,
            tc.tile_pool(name="psA", bufs=2, space="PSUM") as psA,
            tc.tile_pool(name="psB", bufs=1, space="PSUM") as psB,
            tc.tile_pool(name="psC", bufs=2, space="PSUM") as psC,
        ):
            # persistent SBUF
            hT = pp.tile([D, NPC], bf16)
            radj = pp.tile([P, NB * (D + 1)], bf16)
            idxs = pp.tile([P, 8 * S], i16)
            pidx = pp.tile([P, 8 * SG], i16)
            hmask = pp.tile([P, 2 * S], bf16)
            NRt = pp.tile([D, N_LAYERS * (D + 1)], bf16)
            Qst = pp.tile([D, N_LAYERS * D], f32)
            bcol = pp.tile([D, N_LAYERS], f32)
            brow3 = pp.tile([P, D], f32)
            fcwT = pp.tile([D, N_CLASSES], f32)
            fcb = pp.tile([P, N_CLASSES], f32)
            ident = pp.tile([P, P], f32)
            sentP = pp.tile([1, D], f32)
            nc.sync.dma_start(hT[:], t_xT[:])
            nc.sync.dma_start(idxs[:], t_idx[:])
            nc.sync.dma_start(pidx[:], t_pidx[:])
            nc.sync.dma_start(hmask[:], t_hmask[:])
            nc.sync.dma_start(NRt[:], t_NR[:])
            nc.sync.dma_start(Qst[:], t_Qs[:])
            nc.sync.dma_start(bcol[:], t_bcol[:])
            nc.sync.dma_start(brow3[:], t_brow3[:])
            nc.sync.dma_start(fcwT[:], t_fcwT[:])
            nc.sync.dma_start(fcb[:], t_fcb[:])
            make_identity(nc, ident[:])
            nc.vector.memset(sentP[:], -3.0e38)
            nc.sync.dma_start(t_h3loc[NPC:NPC + 1, :], sentP[:])

            tab_pairs = t_tabFull[:].rearrange("(a b) d -> a (b d)", b=2)
            gq = [0]

            for l in range(N_LAYERS):
                sl = float(s_l[l])
                # ---- node pass ----
                for j in range(NB):
                    np_ps = psA.tile([P, D + 1], f32, tag="npp")
                    nc.tensor.matmul(
                        out=np_ps[:],
                        lhsT=hT[:, j * P:(j + 1) * P],
                        rhs=NRt[:, l * (D + 1):(l + 1) * (D + 1)],
                        start=True, stop=True,
                    )
                    nc.scalar.activation(
                        out=radj[:, j * (D + 1):(j + 1) * (D + 1)], in_=np_ps[:],
                        func=AF.Copy)
                    nc.sync.dma_start(
                        t_tabA[j * P:(j + 1) * P, :],
                        radj[:, j * (D + 1):j * (D + 1) + D])
                # ---- exchange ----
                nc.gpsimd.collective_compute(
                    "AllGather", mybir.AluOpType.bypass,
                    replica_groups=[list(range(N_CORES))],
                    ins=[t_tabA[:].opt()],
                    outs=[t_tabFull[:].opt()],
                )
                # ---- self-loop weights for all blocks: w = max(e^e2, e^.2e2)
                r0s = radj[:].rearrange("p (j e) -> p j e", e=D + 1)[:, :, 0:1] \
                    .rearrange("p j one -> p (j one)")
                ads = radj[:].rearrange("p (j e) -> p j e", e=D + 1)[:, :, D:D + 1] \
                    .rearrange("p j one -> p (j one)")
                adc = wp.tile([P, NB], f32, tag="adc")
                nc.scalar.activation(out=adc[:], in_=ads, func=AF.Copy)
                e2s = wp.tile([P, NB], f32, tag="e2s")
                nc.scalar.activation(out=e2s[:], in_=r0s, func=AF.Copy, scale=sl)
                nc.vector.tensor_tensor(out=e2s[:], in0=e2s[:], in1=adc[:], op=OP.add)
                wsA = wp.tile([P, NB], f32, tag="wsA")
                wself = wp.tile([P, NB], f32, tag="wself")
                nc.scalar.activation(out=wsA[:], in_=e2s[:], func=AF.Exp)
                nc.scalar.activation(out=wself[:], in_=e2s[:], func=AF.Exp,
                                     scale=NEG_SLOPE)
                nc.vector.tensor_tensor(out=wself[:], in0=wself[:], in1=wsA[:],
                                        op=OP.max)
                # ---- edge pass per chunk ----
                for (j0, g, Cc, oc) in chunks:
                    ncols = g * Cc
                    gt = gp.tile([P, ncols * P], bf16, tag="gt")
                    gt_g = gt[:].rearrange("p (k e) -> p k e", e=P)
                    npieces = -(-ncols // MAX_GATHER_COLS)
                    a = 0
                    for pi in range(npieces):
                        b = a + (ncols - a) // (npieces - pi)
                        nidx = (b - a) * P
                        nc.gpsimd.dma_gather(
                            out_ap=gt_g[:, a:b, :],
                            in_ap=tab_pairs,
                            idxs_ap=idxs[:, 8 * (oc + a):8 * (oc + b)],
                            num_idxs=nidx,
                            num_idxs_reg=nidx,
                            elem_size=P,
                            single_packet=False,
                            queue_num=gq[0] % 4,
                        )
                        gq[0] += 1
                        a = b
                    # attention weights over [P, 2*ncols]
                    r0 = gt[:].rearrange("p (k e) -> p k e", e=D)[:, :, 0:1] \
                        .rearrange("p k one -> p (k one)")
                    e2 = wp.tile([P, 2 * ncols], f32, tag="e2")
                    nc.scalar.activation(out=e2[:], in_=r0, func=AF.Copy, scale=sl)
                    nc.vector.tensor_tensor(
                        out=e2[:].rearrange("p (g c) -> p g c", g=g),
                        in0=e2[:].rearrange("p (g c) -> p g c", g=g),
                        in1=adc[:, j0:j0 + g].to_broadcast([P, g, 2 * Cc]),
                        op=OP.add)
                    wA = wp.tile([P, 2 * ncols], f32, tag="wA")
                    wB = wp.tile([P, 2 * ncols], f32, tag="wB")
                    nc.scalar.activation(out=wA[:], in_=e2[:], func=AF.Exp)
                    nc.scalar.activation(out=wB[:], in_=e2[:], func=AF.Exp,
                                         scale=NEG_SLOPE)
                    wmb = wp.tile([P, 2 * ncols], bf16, tag="wmb")
                    nc.vector.tensor_tensor(out=wmb[:], in0=wB[:], in1=wA[:],
                                            op=OP.max)
                    wm = wp.tile([P, 2 * ncols], bf16, tag="wm")
                    nc.vector.tensor_tensor(
                        out=wm[:], in0=wmb[:],
                        in1=hmask[:, 2 * oc:2 * (oc + ncols)], op=OP.mult)
                    # denominator per block
                    dn = wp.tile([P, g], f32, tag="dn")
                    nc.vector.reduce_sum(
                        out=dn[:],
                        in_=wm[:].rearrange("p (g c) -> p g c", g=g),
                        axis=mybir.AxisListType.X)
                    nc.vector.tensor_tensor(
                        out=dn[:], in0=dn[:], in1=wself[:, j0:j0 + g], op=OP.add)
                    recip = wp.tile([P, g], f32, tag="recip")
                    nc.vector.reciprocal(out=recip[:], in_=dn[:])
                    # weight the gathered rows (in place, pure bf16)
                    nc.vector.tensor_tensor(
                        out=gt[:].rearrange("p (k e) -> p k e", e=D),
                        in0=gt[:].rearrange("p (k e) -> p k e", e=D),
                        in1=wm[:].to_broadcast([P, 2 * ncols, D]),
                        op=OP.mult)
                    # in-place pairwise tree-sum over the 2*Cc slot axis
                    gt4 = gt[:].rearrange("p (g c d) -> p g c d", g=g, d=D)
                    cw = 2 * Cc
                    while cw > 1:
                        half = cw // 2
                        rem = cw - 2 * half
                        nc.vector.tensor_tensor(
                            out=gt4[:, :, 0:half, :],
                            in0=gt4[:, :, 0:half, :],
                            in1=gt4[:, :, half + rem:cw, :], op=OP.add)
                        cw = half + rem
                    # U = tree + self, normalized
                    selfr = radj[:].rearrange("p (j e) -> p j e", e=D + 1)[
                        :, j0:j0 + g, 0:D]
                    U = wp.tile([P, g * D], f32, tag="U")
                    nc.vector.tensor_tensor(
                        out=U[:].rearrange("p (g d) -> p g d", d=D),
                        in0=selfr,
                        in1=wself[:, j0:j0 + g].to_broadcast([P, g, D]),
                        op=OP.mult)
                    nc.vector.tensor_tensor(
                        out=U[:].rearrange("p (g d) -> p g d", d=D),
                        in0=U[:].rearrange("p (g d) -> p g d", d=D),
                        in1=gt4[:, :, 0:1, :].rearrange("p g one d -> p (g one) d"),
                        op=OP.add)
                    nc.vector.tensor_tensor(
                        out=U[:].rearrange("p (g d) -> p g d", d=D),
                        in0=U[:].rearrange("p (g d) -> p g d", d=D),
                        in1=recip[:].to_broadcast([P, g, D]),
                        op=OP.mult)
                    # transpose blocks into one PSUM tile -> aggT [64, g*128]
                    at_ps = psB.tile([D, g * P], f32, tag="at")
                    for gg in range(g):
                        nc.tensor.transpose(
                            out=at_ps[:, gg * P:(gg + 1) * P],
                            in_=U[:, gg * D:(gg + 1) * D],
                            identity=ident[:])
                    aggT = wp.tile([D, g * P], f32, tag="aggT")
                    nc.scalar.activation(out=aggT[:], in_=at_ps[:], func=AF.Copy)
                    if l < N_LAYERS - 1:
                        h_ps = psB.tile([D, g * P], f32, tag="hps")
                        nc.tensor.matmul(
                            out=h_ps[:], lhsT=Qst[:, l * D:(l + 1) * D],
                            rhs=aggT[:], start=True, stop=True)
                        nc.vector.tensor_scalar(
                            out=hT[:, j0 * P:(j0 + g) * P], in0=h_ps[:],
                            scalar1=bcol[:, l:l + 1], scalar2=0.0,
                            op0=OP.add, op1=OP.max)
                    else:
                        for gg in range(g):
                            h3_ps = psC.tile([P, D], f32, tag="h3ps")
                            nc.tensor.matmul(
                                out=h3_ps[:],
                                lhsT=aggT[:, gg * P:(gg + 1) * P],
                                rhs=Qst[:, l * D:(l + 1) * D],
                                start=True, stop=True)
                            h3 = wp.tile([P, D], f32, tag="h3")
                            nc.vector.tensor_tensor(
                                out=h3[:], in0=h3_ps[:], in1=brow3[:], op=OP.add)
                            nc.sync.dma_start(
                                t_h3loc[(j0 + gg) * P:(j0 + gg + 1) * P, :], h3[:])
            # ---- pooling: segment max over graphs ----
            for q in range(GB):
                PCq = int(PC[q])
                pg_t = pgp.tile([P, PCq * D], f32, tag="pg")
                pg_g = pg_t[:].rearrange("p (k e) -> p k e", e=D)
                a = 0
                while a < PCq:
                    b = min(a + MAX_GATHER_COLS, PCq)
                    nidx = (b - a) * P
                    nc.gpsimd.dma_gather(
                        out_ap=pg_g[:, a:b, :],
                        in_ap=t_h3loc[:],
                        idxs_ap=pidx[:, 8 * (poffs[q] + a):8 * (poffs[q] + b)],
                        num_idxs=nidx,
                        num_idxs_reg=nidx,
                        elem_size=D,
                        single_packet=False,
                        queue_num=gq[0] % 4,
                    )
                    gq[0] += 1
                    a = b
                cw = PCq
                while cw > 1:
                    half = cw // 2
                    rem = cw - 2 * half
                    nc.vector.tensor_tensor(
                        out=pg_g[:, 0:half, :], in0=pg_g[:, 0:half, :],
                        in1=pg_g[:, half + rem:cw, :], op=OP.max)
                    cw = half + rem
                nc.sync.dma_start(t_gpart[q * P:(q + 1) * P, :], pg_t[:, 0:D])
            nc.gpsimd.collective_compute(
                "AllReduce", mybir.AluOpType.max,
                replica_groups=[list(range(N_CORES))],
                ins=[t_gpart[:].opt()],
                outs=[t_gall[:].opt()],
            )
            # ---- fc + log_softmax (redundant on all cores) ----
            for q in range(GB):
                gsb = wp.tile([P, D], f32, tag="gsb")
                nc.sync.dma_start(gsb[:], t_gall[q * P:(q + 1) * P, :])
                mask = wp.tile([P, D], f32, tag="mask")
                nc.vector.tensor_scalar(
                    out=mask[:], in0=gsb[:], scalar1=-1.0e37, scalar2=None,
                    op0=OP.is_gt)
                nc.vector.tensor_tensor(out=gsb[:], in0=gsb[:], in1=mask[:],
                                        op=OP.mult)
                gT_ps = psB.tile([D, P], f32, tag="gT")
                nc.tensor.transpose(out=gT_ps[:], in_=gsb[:], identity=ident[:])
                gT = wp.tile([D, P], f32, tag="gTs")
                nc.vector.tensor_copy(out=gT[:], in_=gT_ps[:])
                lg_ps = psB.tile([P, N_CLASSES], f32, tag="lg")
                nc.tensor.matmul(out=lg_ps[:], lhsT=gT[:], rhs=fcwT[:],
                                 start=True, stop=True)
                lg = wp.tile([P, N_CLASSES], f32, tag="lgs")
                nc.vector.tensor_tensor(
                    out=lg[:], in0=lg_ps[:], in1=fcb[:], op=OP.add)
                m = wp.tile([P, 1], f32, tag="m")
                nc.vector.reduce_max(out=m[:], in_=lg[:], axis=mybir.AxisListType.X)
                mneg = wp.tile([P, 1], f32, tag="mneg")
                nc.vector.tensor_scalar_mul(out=mneg[:], in0=m[:], scalar1=-1.0)
                ex = wp.tile([P, N_CLASSES], f32, tag="ex")
                sumex = wp.tile([P, 1], f32, tag="sumex")
                nc.scalar.activation(out=ex[:], in_=lg[:], func=AF.Exp,
                                     bias=mneg[:], accum_out=sumex[:])
                logz = wp.tile([P, 1], f32, tag="logz")
                nc.scalar.activation(out=logz[:], in_=sumex[:], func=AF.Ln)
                off = wp.tile([P, 1], f32, tag="off")
                nc.vector.tensor_add(out=off[:], in0=m[:], in1=logz[:])
                outsb = wp.tile([P, N_CLASSES], f32, tag="outsb")
                nc.vector.tensor_tensor(
                    out=outsb[:], in0=lg[:],
                    in1=off[:].to_broadcast([P, N_CLASSES]), op=OP.subtract)
                nc.sync.dma_start(t_out[q * P:(q + 1) * P, :], outsb[:])
    nc.compile()
    return nc


def kernel(**inputs):
    x = np.asarray(inputs["x"])
    key = (x.shape, inputs["edge_index"].shape)
    host, per_core = _host_prep(**inputs)
    if key not in _COMPILED:
        _COMPILED[key] = _build(host)
    nc = _COMPILED[key]
    in_maps = [per_core[c] for c in range(N_CORES)]
    import os
    trace = False
    if os.environ.get("KERNEL_TRACE") == "1":
        try:
            import types
            if "antenv.axon_hooks" not in sys.modules:
                import antenv
                from trn_agent_boot.trn_boot import _ntff_profile_via_ctypes
                mod = types.ModuleType("antenv.axon_hooks")
                _state = {"hook": _ntff_profile_via_ctypes("/opt/axon/libaxon_pjrt.so")}
                mod.set_axon_ntff_profile_hook = lambda h: _state.__setitem__("hook", h)
                mod.get_axon_ntff_profile_hook = lambda: _state["hook"]
                sys.modules["antenv.axon_hooks"] = mod
                antenv.axon_hooks = mod
            trace = True
        except Exception:
            trace = False
    res = bass_utils.run_bass_kernel_spmd(
        nc, in_maps, core_ids=list(range(N_CORES)), trace=trace)
    globals()['LAST_EXEC_NS'] = res.exec_time_ns
    raw = np.asarray(res.results[0]["out"], np.float32)
    return raw[host["slot_of_graph"]]


LAST_EXEC_NS = None
